# revision 32
# baseline (speedup 1.0000x reference)
"""Fake-attention kernel for trn2: 8 NeuronCores, one batch element per core.

Per core (batch b): out = softmax(k @ q^T) @ v, with k/q/v = x @ W.T + b.

Dataflow (transposed so the PV contraction lands on partitions):
  xT [f,n]     <- transposed on host; loaded via gpsimd casting DMAs so
                  the tiles carry fp32r (full-rate PE) directly
  kT,qT [d,n]  = W @ xT (fp32r) + bias at the PSUM->SBUF copy-out
                 (setup adds on ACT, prologue on DVE)
  v [m,d]      = xT-chunks as lhsT, rhs = Wv^T (fp32r); the copy-out adds
                 bv broadcast along d and writes BF16 (bv rides the PV sum
                 exactly because softmax weights sum to 1)
  per n-section of 1024, m-chunks of 128 (PV lags scores by 3 chunks;
  the 3-deep PSUM scores pool means the exp of chunk c never gates the
  scores matmul of chunk c+2 — the exp->scores chain that capped the
  2-buffer design is gone):
    scoresT chunk [m=128, n=1024] = qT-slice as lhsT, kT as rhs (fp32r)
    pT = exp(scoresT): chunks mc%4==1 on DVE as a Schraudolph bf16 bit
    pattern (single fused mult+add, int16 convert-on-write), the rest on
    ACT (PSUM->SBUF, bf16 convert-on-write); each scores tile has exactly
    one reader (PSUM tiles serialize readers)
    outT [d,n] += v-chunk(bf16) as lhsT, pT(bf16) as rhs (PSUM acc, one
    shared accumulator buffer - the copy-out below frees it in time)
    denominator: column split - DVE adds cols [0:dcol] in BF16 (2x_1p,
    two ping-pong accumulators for precision), GPSIMD adds cols
    [dcol:1024] (also BF16 pairs; GPSIMD cost is dtype-blind)
  finalize (spread over the NEXT section's chunk stream):
    og = PSUM outT -> SBUF f32 (ACT half / DVE half; frees the acc buf)
    denom row [1,1024] via ones-matmuls from the bf16 accumulators into a
    borrowed scores tile -> DVE reciprocal row -> GPSIMD
    partition_broadcast -> og *= rdbc (two DVE sbuf tensor_tensor mults)
    DMA halves straight out in [d, n] orientation over the SP/ACT queues
    (y is declared transposed [D, N]; the host transposes back).

PSUM (8 banks): scores pool 3x[128,1024]f32 (6 banks) + acc 1x (2).
Setup projections (k/q/v) borrow scores-pool tiles; the denominator row
borrows one for two steps of the finalize.
"""
import numpy as np

B = 8
N = 4096
D = 128
NC = 32          # chunks of 128 along n/m
NSEC = 4         # sections of 1024 along n
SEC = 1024
DCOL = 704       # denominator cols on DVE (d_acc); rest on GPSIMD (d_odd)
SCH_A = 184.6646884   # 2^7 * log2(e): Schraudolph exp in bf16 bit space
SCH_B = 16248.0       # (127 - c) * 2^7, calibrated on the real score range

_cache = {}


def _build(dcol=DCOL, warmup_mms=16, ptp_bufs=7):
    import concourse.bass as bass  # noqa
    import concourse.mybir as mybir
    import concourse.tile as tile
    from concourse import bacc

    F32 = mybir.dt.float32
    F32R = mybir.dt.float32r
    BF16 = mybir.dt.bfloat16
    I16 = mybir.dt.int16
    Exp = mybir.ActivationFunctionType.Exp
    Ident = mybir.ActivationFunctionType.Identity
    ADD = mybir.AluOpType.add
    MULT = mybir.AluOpType.mult
    ecol = SEC - dcol
    PVLAG = 3

    nc = bacc.Bacc()
    xt = nc.declare_dram_parameter("xt", [D, N], F32, isOutput=False)
    xb = nc.declare_dram_parameter("xb", [D, N], BF16, isOutput=False)
    wp = nc.declare_dram_parameter("wp", [128, 643], F32, isOutput=False)
    y = nc.declare_dram_parameter("y", [D, N], F32, isOutput=True)

    xt_dram = xt.rearrange("p (c l) -> p c l", l=128)
    xb_dram = xb.rearrange("p (c l) -> p c l", l=128)

    with tile.TileContext(nc) as tc:
        with (
            tc.tile_pool(name="big", bufs=1) as big,
            tc.tile_pool(name="ptp", bufs=ptp_bufs) as ptp,
            tc.tile_pool(name="wrk", bufs=2) as wrk,
            tc.tile_pool(name="sc", bufs=3, space="PSUM") as ps_sc,
            tc.tile_pool(name="acc", bufs=1, space="PSUM") as ps_acc,
        ):
            # --- input DMAs, criticality-ordered -------------------------
            xg0a = big.tile([128, 4, 128], F32R, tag="xT0a")
            xg0b = big.tile([128, 4, 128], F32R, tag="xT0b")
            wf32 = big.tile([128, 512], F32, tag="wf32")
            wbig = big.tile([128, 512], F32R, tag="wbig")
            wrest = big.tile([128, 131], F32, tag="wrest")
            wsm = big.tile([128, 130], BF16, tag="wsm")
            nc.gpsimd.dma_start(xg0a[:], xt_dram[:, 0:4, :])
            nc.gpsimd.dma_start(xg0b[:], xt_dram[:, 4:8, :])
            nc.sync.dma_start(wf32[:, 0:256], wp[:, 0:256])
            nc.sync.dma_start(wrest[:], wp[:, 512:643])
            nc.sync.dma_start(wf32[:, 256:512], wp[:, 256:512])
            # rounding copies: critical wq/wk now; wv after the prologue's
            # k-chain (only needed for v0)
            nc.vector.tensor_copy(wbig[:, 128:256], wf32[:, 128:256])
            nc.vector.tensor_copy(wbig[:, 0:128], wf32[:, 0:128])
            wkT = wbig[:, 0:128]
            wqT = wbig[:, 128:256]
            wvT = wbig[:, 256:384]
            ones_bf = wsm[:, 0:1]
            wvT_bf = wsm[:, 2:130]
            bv_bc = wrest[:, 1:129]
            bk = wrest[:, 129:130]
            bq = wrest[:, 130:131]

            xT_g = [None] * 4
            xT_g[0] = (xg0a, xg0b)
            xB_g = [None] * 4
            xb0 = big.tile([128, 8, 128], BF16, tag="xB0")
            nc.sync.dma_start(xb0[:], xb_dram[:, 0:8, :])
            xB_g[0] = xb0

            def emit_dma_x(g):
                xg = big.tile([128, 8, 128], F32R, tag=f"xT{g}")
                nc.gpsimd.dma_start(xg[:], xt_dram[:, g * 8:(g + 1) * 8, :])
                xT_g[g] = xg
                xbg = big.tile([128, 8, 128], BF16, tag=f"xB{g}")
                nc.sync.dma_start(xbg[:], xb_dram[:, g * 8:(g + 1) * 8, :])
                xB_g[g] = xbg

            emit_dma_x(1)

            wu = big.tile([128, 128], BF16, tag="warm")
            nc.vector.memset(wu[:], 1.0)
            wu_ps = ps_sc.tile([128, 1024], F32, tag="sc")
            for _ in range(warmup_mms):
                nc.tensor.matmul(wu_ps[:, 0:128], wu[:], wu[:],
                                 start=True, stop=True,
                                 skip_group_check=True)

            def sctile():
                t = ps_sc.tile([128, 1024], F32, tag="sc")
                return t

            def xslab(g, half):
                xg = xT_g[g]
                if isinstance(xg, tuple):
                    return xg[half].rearrange("p c f -> p (c f)")
                return xg.rearrange("p c f -> p (c f)")[
                    :, half * 512:(half + 1) * 512]

            def xchunk(g, j):
                xg = xT_g[g]
                if isinstance(xg, tuple):
                    return xg[j // 4][:, j % 4, :]
                return xg[:, j, :]

            kT = [None] * 4
            qT = [None] * 4
            v_g = [None] * 4

            # --- projection helpers (psum borrowed from the scores pool) -
            def emit_k_half(g, half):
                if half == 0:
                    tg = big.tile([128, 1024], F32R, tag=f"kT{g}")
                    kT[g] = tg
                else:
                    tg = kT[g]
                pst = sctile()
                lo, hi = half * 512, half * 512 + 512
                nc.tensor.matmul(pst[:, 0:512], wkT, xslab(g, half),
                                 start=True, stop=True)
                nc.scalar.activation(tg[:, lo:hi], pst[:, 0:512], Ident,
                                     bias=bk)

            def v_bias_copy(vg, psv3, lo, hi):
                n = hi - lo
                bvx = bv_bc[:, None, :].to_broadcast((128, n, 128))
                nc.vector.tensor_tensor(
                    vg[:, lo:hi, :], psv3[:, lo:hi, :], bvx, ADD)

            def xbchunk(g, j):
                return xB_g[g][:, j, :]

            def emit_v(g):
                vg = big.tile([128, 8, 128], BF16, tag=f"v{g}")
                psv = sctile()
                psv3 = psv.rearrange("p (c f) -> p c f", f=128)
                for j in range(8):
                    nc.tensor.matmul(
                        psv[:, j * 128:(j + 1) * 128], xbchunk(g, j), wvT_bf,
                        start=True, stop=True,
                    )
                v_bias_copy(vg, psv3, 0, 8)
                v_g[g] = vg

            def q_slice(mc):
                return qT[mc // 8][:, (mc % 8) * 128:(mc % 8 + 1) * 128]

            def v_chunk(mc):
                return v_g[mc // 8][:, mc % 8, :]

            # --- prologue ------------------------------------------------
            # q0[0:512] rushed into the warmup tile's spare columns
            qt0 = big.tile([128, 1024], F32R, tag="qT0")
            qT[0] = qt0
            nc.tensor.matmul(wu_ps[:, 512:640], wqT,
                             xslab(0, 0)[:, 0:128], start=True, stop=True)
            nc.vector.tensor_scalar_add(qt0[:, 0:128], wu_ps[:, 512:640], bq)
            nc.tensor.matmul(wu_ps[:, 640:1024], wqT,
                             xslab(0, 0)[:, 128:512], start=True, stop=True)
            # k0
            kga = big.tile([128, 512], F32R, tag="kT0a")
            kgb = big.tile([128, 512], F32R, tag="kT0b")
            k0ps = sctile()
            nc.tensor.matmul(k0ps[:, 0:512], wkT, xslab(0, 0),
                             start=True, stop=True)
            nc.vector.tensor_scalar_add(kga[:], k0ps[:, 0:512], bk)
            kT[0] = (kga, kgb)
            # scores chunk 0 first half + exp
            q_sl0 = q_slice(0)
            s0 = sctile()
            pT0 = ptp.tile([128, 1024], I16, tag="pt")
            pT0b = pT0.bitcast(BF16)
            nc.tensor.matmul(s0[:, 0:512], q_sl0, kga[:], start=True, stop=True)
            nc.scalar.activation(pT0b[:, 0:512], s0[:, 0:512], Exp)
            # k0 second half + scores 0 second half
            nc.tensor.matmul(k0ps[:, 512:1024], wkT, xslab(0, 1),
                             start=True, stop=True)
            nc.vector.tensor_scalar_add(kgb[:], k0ps[:, 512:1024], bk)
            nc.tensor.matmul(s0[:, 512:1024], q_sl0, kgb[:], start=True, stop=True)
            nc.scalar.activation(pT0b[:, 512:1024], s0[:, 512:1024], Exp)
            # deferred weight casts + v0 + q0 tail
            nc.vector.tensor_copy(wbig[:, 256:384], wf32[:, 256:384])
            nc.vector.tensor_copy(wvT_bf[:], wf32[:, 256:384])
            nc.vector.memset(ones_bf, 1.0)
            orow_f = big.tile([1, 128], F32, tag="orf")
            orow = big.tile([1, 128], F32R, tag="oro")
            nc.vector.memset(orow_f[:], 1.0)
            nc.vector.tensor_copy(orow[:], orow_f[:])
            emit_v(0)
            nc.vector.tensor_scalar_add(qt0[:, 128:512], wu_ps[:, 640:1024], bq)
            q0ps = sctile()
            nc.tensor.matmul(q0ps[:, 512:1024], wqT, xslab(0, 1),
                             start=True, stop=True)
            nc.vector.tensor_scalar_add(qt0[:, 512:1024], q0ps[:, 512:1024], bq)

            fin_jobs = [None]
            setup_sched = {}

            def add_setup(mc, fn):
                setup_sched.setdefault(mc, []).append(fn)

            setup_state = {}

            def emit_q_mm(g):
                tg = big.tile([128, 1024], F32R, tag=f"qT{g}")
                pst = sctile()
                nc.tensor.matmul(pst[:, 0:512], wqT, xslab(g, 0),
                                 start=True, stop=True)
                nc.tensor.matmul(pst[:, 512:1024], wqT, xslab(g, 1),
                                 start=True, stop=True)
                setup_state[f"q{g}"] = (tg, pst)
                qT[g] = tg

            def emit_q_add(g):
                tg, pst = setup_state.pop(f"q{g}")
                nc.scalar.activation(tg[:], pst[:], Ident, bias=bq)

            def emit_k_mm(g):
                tg = big.tile([128, 1024], F32R, tag=f"kT{g}")
                pst = sctile()
                nc.tensor.matmul(pst[:, 0:512], wkT, xslab(g, 0),
                                 start=True, stop=True)
                nc.tensor.matmul(pst[:, 512:1024], wkT, xslab(g, 1),
                                 start=True, stop=True)
                setup_state[f"k{g}"] = (tg, pst)
                kT[g] = tg

            def emit_k_add(g):
                tg, pst = setup_state.pop(f"k{g}")
                nc.scalar.activation(tg[:], pst[:], Ident, bias=bk)

            def emit_v_mm(g, quarter):
                if quarter == 0:
                    vg = big.tile([128, 8, 128], BF16, tag=f"v{g}")
                    psv = sctile()
                    setup_state[f"v{g}"] = (vg, psv)
                    v_g[g] = vg
                else:
                    vg, psv = setup_state[f"v{g}"]
                for j in range(quarter * 2, quarter * 2 + 2):
                    nc.tensor.matmul(
                        psv[:, j * 128:(j + 1) * 128], xbchunk(g, j), wvT_bf,
                        start=True, stop=True,
                    )

            def emit_v_copy(g):
                vg, psv = setup_state.pop(f"v{g}")
                psv3 = psv.rearrange("p (c f) -> p c f", f=128)
                v_bias_copy(vg, psv3, 0, 8)

            add_setup(2, lambda: emit_q_mm(1))
            add_setup(2, lambda: emit_dma_x(2))
            add_setup(4, lambda: emit_q_add(1))
            add_setup(4, lambda: emit_dma_x(3))
            for s, qq in ((4, 0), (5, 1), (6, 2), (7, 3)):
                add_setup(s, lambda q=qq: emit_v_mm(1, q))
            add_setup(8, lambda: emit_v_copy(1))
            add_setup(10, lambda: emit_q_mm(2))
            add_setup(12, lambda: emit_q_add(2))
            for s, qq in ((12, 0), (13, 1), (14, 2), (15, 3)):
                add_setup(s, lambda q=qq: emit_v_mm(2, q))
            add_setup(16, lambda: emit_v_copy(2))
            add_setup(18, lambda: emit_q_mm(3))
            add_setup(20, lambda: emit_q_add(3))
            for s, qq in ((20, 0), (21, 1), (22, 2), (23, 3)):
                add_setup(s, lambda q=qq: emit_v_mm(3, q))
            add_setup(24, lambda: emit_v_copy(3))
            add_setup(26, lambda: emit_k_mm(1))
            add_setup(28, lambda: emit_k_add(1))

            pT_prev = pT0
            last_pv_chain = [None]

            for sec in range(NSEC):
                d_acc_a = wrk.tile([128, dcol], BF16, tag="dea")
                d_acc_b = wrk.tile([128, dcol], BF16, tag="deb")
                d_acc = [d_acc_a, d_acc_b]
                d_odd_a = wrk.tile([128, ecol], BF16, tag="doa")
                d_odd_b = wrk.tile([128, ecol], BF16, tag="dob")
                d_odd = [d_odd_a, d_odd_b]
                ps_pv_t = ps_acc.tile([128, 1024], F32, tag="acct")

                def emit_scores(mc, sec=sec):
                    ps_s = sctile()
                    q_sl = q_slice(mc)
                    kg = kT[sec]
                    if isinstance(kg, tuple):
                        ka, kb = kg[0][:], kg[1][:]
                    else:
                        ka, kb = kg[:, 0:512], kg[:, 512:1024]
                    nc.tensor.matmul(ps_s[:, 0:512], q_sl, ka,
                                     start=True, stop=True)
                    nc.tensor.matmul(ps_s[:, 512:1024], q_sl, kb,
                                     start=True, stop=True)
                    return ps_s

                def emit_exp(ps_s, mc):
                    # one reader per PSUM scores tile: whole-chunk exp on
                    # DVE (Schraudolph, mc%4==1) or ACT (exact), a single
                    # instruction either way
                    pT = ptp.tile([128, 1024], I16, tag="pt")
                    if mc % 4 == 1:
                        nc.vector.tensor_scalar(
                            pT[:], ps_s[:], SCH_A, SCH_B, MULT, ADD)
                    else:
                        nc.scalar.activation(pT.bitcast(BF16)[:], ps_s[:], Exp)
                    return pT

                def emit_pv(mc, pT, ps_pv_t=ps_pv_t):
                    pTb = pT.bitcast(BF16)
                    nc.tensor.matmul(
                        ps_pv_t[:, 0:512], v_chunk(mc), pTb[:, 0:512],
                        start=(mc == 0), stop=(mc == NC - 1),
                        skip_group_check=True,
                    )
                    nc.tensor.matmul(
                        ps_pv_t[:, 512:1024], v_chunk(mc), pTb[:, 512:1024],
                        start=(mc == 0), stop=(mc == NC - 1),
                        skip_group_check=True,
                    )

                def emit_denom(mc, pT, d_odd=d_odd, d_acc=d_acc, sec=sec):
                    pTb = pT.bitcast(BF16)
                    da = d_acc[mc % 2]
                    if mc < 2:
                        nc.vector.tensor_copy(da[:], pTb[:, 0:dcol])
                    else:
                        nc.vector.tensor_tensor(
                            da[:], da[:], pTb[:, 0:dcol], ADD)
                    do = d_odd[mc % 2]
                    if mc < 2:
                        nc.gpsimd.tensor_copy(do[:], pTb[:, dcol:SEC])
                    else:
                        nc.gpsimd.tensor_tensor(
                            do[:], do[:], pTb[:, dcol:SEC], ADD)

                # finalize of THIS section, spread over the next section:
                # og copy-out (frees acc) -> denom row (ones-matmuls into a
                # borrowed scores tile) -> reciprocal row -> gpsimd
                # partition broadcast -> og *= rdbc -> straight DMA out
                def make_fin(sec=sec, ps_pv_t=ps_pv_t,
                             d_odd=d_odd, d_acc=d_acc, pT31=None):
                    st = {}
                    last = (sec == NSEC - 1)

                    def f_ocopy(half):
                        if "og" not in st:
                            og = big.tile([128, 1024], F32,
                                          tag=f"out{sec % 2}")
                            st["og"] = og
                        lo, hi = half * 512, half * 512 + 512
                        if half == 0:
                            nc.scalar.copy(st["og"][:, lo:hi],
                                           ps_pv_t[:, lo:hi])
                        else:
                            nc.vector.tensor_copy(st["og"][:, lo:hi],
                                                  ps_pv_t[:, lo:hi])

                    def f_rowmm():
                        ftr = sctile()
                        st["ftr"] = ftr
                        row = ftr[0:1, :]
                        for i in range(2):
                            nc.tensor.matmul(
                                row[:, 0:512], ones_bf, d_acc[i][:, 0:512],
                                start=(i == 0), stop=(i == 1),
                                skip_group_check=True)
                        for i in range(2):
                            nc.tensor.matmul(
                                row[:, 512:dcol], ones_bf,
                                d_acc[i][:, 512:dcol],
                                start=(i == 0), stop=(i == 1),
                                skip_group_check=True)
                        for i in range(2):
                            nc.tensor.matmul(
                                row[:, dcol:SEC], ones_bf, d_odd[i][:],
                                start=(i == 0), stop=(i == 1),
                                skip_group_check=True)

                    def f_recip():
                        rd = wrk.tile([1, 1024], F32, tag="rdr")
                        st["rd"] = rd
                        nc.vector.reciprocal(rd[:], st["ftr"][0:1, :])

                    def f_bc():
                        rdbc = wrk.tile([128, 1024], F32, tag="rdbc")
                        st["rdbc"] = rdbc
                        nc.gpsimd.partition_broadcast(rdbc[:], st["rd"][:])

                    def mk_scale(half):
                        def f_scale():
                            lo, hi = half * 512, half * 512 + 512
                            nc.vector.tensor_tensor(
                                st["og"][:, lo:hi], st["og"][:, lo:hi],
                                st["rdbc"][:, lo:hi], MULT)
                        return f_scale

                    def mk_dma(half, eng):
                        def f_dma():
                            lo, hi = half * 512, half * 512 + 512
                            eng().dma_start(
                                y[:, sec * SEC + lo:sec * SEC + hi],
                                st["og"][:, lo:hi],
                            )
                        return f_dma

                    def f_rowmm_h(half):
                        if half == 0:
                            ftr = sctile()
                            st["ftr"] = ftr
                        row = st["ftr"][0:1, :]
                        p31 = pT31.bitcast(BF16)
                        if half == 0:
                            for i in range(2):
                                nc.tensor.matmul(
                                    row[:, 0:512], ones_bf,
                                    d_acc[i][:, 0:512],
                                    start=(i == 0), stop=False,
                                    skip_group_check=True)
                            nc.tensor.matmul(
                                row[:, 0:512], ones_bf, p31[:, 0:512],
                                start=False, stop=True,
                                skip_group_check=True)
                        else:
                            for i in range(2):
                                nc.tensor.matmul(
                                    row[:, 512:dcol], ones_bf,
                                    d_acc[i][:, 512:dcol],
                                    start=(i == 0), stop=False,
                                    skip_group_check=True)
                            nc.tensor.matmul(
                                row[:, 512:dcol], ones_bf, p31[:, 512:dcol],
                                start=False, stop=True,
                                skip_group_check=True)
                            for i in range(2):
                                nc.tensor.matmul(
                                    row[:, dcol:SEC], ones_bf, d_odd[i][:],
                                    start=(i == 0), stop=False,
                                    skip_group_check=True)
                            nc.tensor.matmul(
                                row[:, dcol:SEC], ones_bf, p31[:, dcol:SEC],
                                start=False, stop=True,
                                skip_group_check=True)

                    def f_recip_h(half):
                        if half == 0:
                            rd = wrk.tile([1, 1024], F32R, tag="rdr2")
                            st["rd"] = rd
                        lo, hi = half * 512, half * 512 + 512
                        with nc.allow_low_precision(
                                reason="f32r reciprocal row for PE bc"):
                            nc.vector.reciprocal(st["rd"][:, lo:hi],
                                                 st["ftr"][0:1, lo:hi])

                    def f_bc_h(half):
                        if half == 0:
                            rdbc = sctile()
                            st["rdbc"] = rdbc
                        lo, hi = half * 512, half * 512 + 512
                        nc.tensor.matmul(
                            st["rdbc"][:, lo:hi], orow, st["rd"][:, lo:hi],
                            start=True, stop=True, skip_group_check=True)

                    if not last:
                        steps = [lambda: f_ocopy(0), lambda: f_ocopy(1),
                                 f_rowmm, f_recip, f_bc,
                                 mk_scale(0), mk_dma(0, lambda: nc.sync),
                                 mk_scale(1), mk_dma(1, lambda: nc.scalar)]
                        return steps
                    # last section: pipelined halves so the first DMA
                    # leaves while the second half still normalizes
                    steps = [lambda: f_ocopy(0),
                             lambda: f_rowmm_h(0), lambda: f_recip_h(0),
                             lambda: f_bc_h(0),
                             lambda: f_ocopy(1),
                             mk_scale(0), mk_dma(0, lambda: nc.sync),
                             lambda: f_rowmm_h(1), lambda: f_recip_h(1),
                             lambda: f_bc_h(1),
                             mk_scale(1), mk_dma(1, lambda: nc.scalar)]
                    return steps

                fin_positions = [2, 3, 4, 5, 6, 7, 8, 9, 10]

                pT_hist = {}
                if sec == 0:
                    pT_hist[0] = pT_prev
                for mc in range(NC):
                    if not (mc == 0 and sec == 0):
                        ps_s = emit_scores(mc)
                    if mc < PVLAG and last_pv_chain[0]:
                        last_pv_chain[0][mc]()
                    if mc >= PVLAG:
                        emit_pv(mc - PVLAG, pT_hist.pop(mc - PVLAG))
                    if mc >= 1:
                        emit_denom(mc - 1, pT_hist[mc - 1])
                    if not (mc == 0 and sec == 0):
                        pT_hist[mc] = emit_exp(ps_s, mc)
                    if mc >= 1:
                        if sec == 0 and mc in setup_sched:
                            for fn in setup_sched[mc]:
                                fn()
                        if sec > 0 and fin_jobs[0]:
                            if mc in fin_positions:
                                idx = fin_positions.index(mc)
                                if idx < len(fin_jobs[0]):
                                    fin_jobs[0][idx]()
                        if sec == 1 and mc == 18:
                            emit_k_half(2, 0)
                        elif sec == 1 and mc == 22:
                            emit_k_half(2, 1)
                        elif sec == 2 and mc == 18:
                            emit_k_half(3, 0)
                        elif sec == 2 and mc == 22:
                            emit_k_half(3, 1)

                # pend the last PVLAG PVs + last denom into the next section
                def mk_last(ps=[pT_hist[NC - PVLAG + i] for i in range(PVLAG)],
                            pv=emit_pv, dn=emit_denom, sec=sec):
                    jobs = []
                    for i in range(PVLAG):
                        def run(i=i):
                            pv(NC - PVLAG + i, ps[i])
                            if i == PVLAG - 1 and sec != NSEC - 1:
                                dn(NC - 1, ps[i])
                        jobs.append(run)
                    return jobs

                last_pv_chain[0] = mk_last()
                fin_jobs[0] = make_fin(pT31=pT_hist[NC - 1])

            # drain: last section's PVs + finalize immediately
            if last_pv_chain[0] is not None:
                for fn in last_pv_chain[0]:
                    fn()
                last_pv_chain[0] = None
            for step in fin_jobs[0]:
                step()

    nc.finalize()
    return nc


def _get_nc():
    if "nc" not in _cache:
        _cache["nc"] = _build()
    return _cache["nc"]


def make_wp(Wk, Wq, Wv, bk, bq, bv):
    # layout: startup-critical wk/wq first so their DMA can land alone
    wp = np.zeros((128, 643), np.float32)
    wp[:, 0:128] = Wk.T
    wp[:, 128:256] = Wq.T
    wp[:, 256:384] = Wv.T
    wp[:, 384:512] = np.eye(128, dtype=np.float32)
    wp[:, 512] = 1.0
    wp[:, 513:641] = np.broadcast_to(bv[None, :], (128, 128))
    wp[:, 641] = bk
    wp[:, 642] = bq
    return wp


def kernel(x, Wk, bk, Wq, bq, Wv, bv, **_ignored):
    from concourse.bass_utils import run_bass_kernel_spmd

    x = np.asarray(x, dtype=np.float32)
    wp = make_wp(
        np.asarray(Wk, np.float32), np.asarray(Wq, np.float32),
        np.asarray(Wv, np.float32), np.asarray(bk, np.float32),
        np.asarray(bq, np.float32), np.asarray(bv, np.float32),
    )

    nc = _get_nc()
    import ml_dtypes
    in_maps = [
        {"xt": np.ascontiguousarray(x[b].T),
         "xb": np.ascontiguousarray(x[b].T).astype(ml_dtypes.bfloat16),
         "wp": wp} for b in range(B)
    ]
    res = run_bass_kernel_spmd(nc, in_maps, core_ids=list(range(B)))
    out = np.stack(
        [np.ascontiguousarray(res.results[b]["y"].T) for b in range(B)],
        axis=0)
    return out


# revision 39
# speedup vs baseline: 1.0023x; 1.0023x over previous
"""Fake-attention kernel for trn2: 8 NeuronCores, one batch element per core.

Per core (batch b): out = softmax(k @ q^T) @ v, with k/q/v = x @ W.T + b.

Dataflow (transposed so the PV contraction lands on partitions):
  xT [f,n]     <- transposed on host; loaded via gpsimd casting DMAs so
                  the tiles carry fp32r (full-rate PE) directly
  kT,qT [d,n]  = W @ xT (fp32r) + bias at the PSUM->SBUF copy-out
                 (setup adds on ACT, prologue on DVE)
  v [m,d]      = xb-chunks (host-prerounded bf16 x) as lhsT, rhs = Wv^T
                 in bf16 (full PE rate at 128 cols); the copy-out adds bv
                 broadcast along d and writes BF16 (bv rides the PV sum
                 exactly because softmax weights sum to 1)
  per n-section of 1024, m-chunks of 128 (PV lags scores by 3 chunks;
  the 3-deep PSUM scores pool means the exp of chunk c never gates the
  scores matmul of chunk c+2 — the exp->scores chain that capped the
  2-buffer design is gone):
    scoresT chunk [m=128, n=1024] = qT-slice as lhsT, kT as rhs (fp32r)
    pT = exp(scoresT): chunks mc%4==1 on DVE as a Schraudolph bf16 bit
    pattern (single fused mult+add, int16 convert-on-write), the rest on
    ACT (PSUM->SBUF, bf16 convert-on-write); each scores tile has exactly
    one reader (PSUM tiles serialize readers)
    outT [d,n] += v-chunk(bf16) as lhsT, pT(bf16) as rhs (PSUM acc, one
    shared accumulator buffer - the copy-out below frees it in time)
    denominator: column split - DVE adds cols [0:dcol] in BF16 (2x_1p,
    two ping-pong accumulators for precision), GPSIMD adds cols
    [dcol:1024] (also BF16 pairs; GPSIMD cost is dtype-blind)
  finalize (spread over the NEXT section's chunk stream):
    og = PSUM outT -> SBUF f32 (ACT half / DVE half; frees the acc buf)
    denom row [1,1024] via ones-matmuls from the bf16 accumulators into a
    borrowed scores tile -> DVE reciprocal row -> GPSIMD
    partition_broadcast -> og *= rdbc (two DVE sbuf tensor_tensor mults)
    DMA halves straight out in [d, n] orientation over the SP/ACT queues
    (y is declared transposed [D, N]; the host transposes back).

PSUM (8 banks): scores pool 3x[128,1024]f32 (6 banks) + acc 1x (2).
Setup projections (k/q/v) borrow scores-pool tiles; the denominator row
borrows one for two steps of the finalize.
"""
import numpy as np

B = 8
N = 4096
D = 128
NC = 32          # chunks of 128 along n/m
NSEC = 4         # sections of 1024 along n
SEC = 1024
DCOL = 704       # denominator cols on DVE (d_acc); rest on GPSIMD (d_odd)
SCH_A = 184.6646884   # 2^7 * log2(e): Schraudolph exp in bf16 bit space
SCH_B = 16248.0       # (127 - c) * 2^7, calibrated on the real score range

_cache = {}


def _build(dcol=DCOL, warmup_mms=16, ptp_bufs=7):
    import concourse.bass as bass  # noqa
    import concourse.mybir as mybir
    import concourse.tile as tile
    from concourse import bacc

    F32 = mybir.dt.float32
    F32R = mybir.dt.float32r
    BF16 = mybir.dt.bfloat16
    I16 = mybir.dt.int16
    Exp = mybir.ActivationFunctionType.Exp
    Ident = mybir.ActivationFunctionType.Identity
    ADD = mybir.AluOpType.add
    MULT = mybir.AluOpType.mult
    ecol = SEC - dcol
    PVLAG = 3

    nc = bacc.Bacc()
    xt = nc.declare_dram_parameter("xt", [D, N], F32, isOutput=False)
    xb = nc.declare_dram_parameter("xb", [D, N], BF16, isOutput=False)
    wp = nc.declare_dram_parameter("wp", [128, 643], F32, isOutput=False)
    y = nc.declare_dram_parameter("y", [D, N], F32, isOutput=True)

    xt_dram = xt.rearrange("p (c l) -> p c l", l=128)
    xb_dram = xb.rearrange("p (c l) -> p c l", l=128)

    with tile.TileContext(nc) as tc:
        with (
            tc.tile_pool(name="big", bufs=1) as big,
            tc.tile_pool(name="ptp", bufs=ptp_bufs) as ptp,
            tc.tile_pool(name="wrk", bufs=2) as wrk,
            tc.tile_pool(name="sc", bufs=3, space="PSUM") as ps_sc,
            tc.tile_pool(name="acc", bufs=1, space="PSUM") as ps_acc,
        ):
            # --- input DMAs, criticality-ordered -------------------------
            xg0a = big.tile([128, 4, 128], F32R, tag="xT0a")
            xg0b = big.tile([128, 4, 128], F32R, tag="xT0b")
            wf32 = big.tile([128, 512], F32, tag="wf32")
            wbig = big.tile([128, 512], F32R, tag="wbig")
            wrest = big.tile([128, 131], F32, tag="wrest")
            wsm = big.tile([128, 130], BF16, tag="wsm")
            nc.gpsimd.dma_start(xg0a[:], xt_dram[:, 0:4, :])
            nc.gpsimd.dma_start(xg0b[:], xt_dram[:, 4:8, :])
            nc.sync.dma_start(wf32[:, 0:256], wp[:, 0:256])
            nc.sync.dma_start(wrest[:], wp[:, 512:643])
            nc.sync.dma_start(wf32[:, 256:512], wp[:, 256:512])
            # rounding copies: critical wq/wk now; wv after the prologue's
            # k-chain (only needed for v0)
            nc.vector.tensor_copy(wbig[:, 128:256], wf32[:, 128:256])
            nc.vector.tensor_copy(wbig[:, 0:128], wf32[:, 0:128])
            wkT = wbig[:, 0:128]
            wqT = wbig[:, 128:256]
            wvT = wbig[:, 256:384]
            ones_bf = wsm[:, 0:1]
            wvT_bf = wsm[:, 2:130]
            bv_bc = wrest[:, 1:129]
            bk = wrest[:, 129:130]
            bq = wrest[:, 130:131]

            xT_g = [None] * 4
            xT_g[0] = (xg0a, xg0b)
            xB_g = [None] * 4
            xb0 = big.tile([128, 8, 128], BF16, tag="xB0")
            nc.sync.dma_start(xb0[:], xb_dram[:, 0:8, :])
            xB_g[0] = xb0

            def emit_dma_x(g):
                xg = big.tile([128, 8, 128], F32R, tag=f"xT{g}")
                nc.gpsimd.dma_start(xg[:], xt_dram[:, g * 8:(g + 1) * 8, :])
                xT_g[g] = xg
                xbg = big.tile([128, 8, 128], BF16, tag=f"xB{g}")
                nc.sync.dma_start(xbg[:], xb_dram[:, g * 8:(g + 1) * 8, :])
                xB_g[g] = xbg

            emit_dma_x(1)

            wu = big.tile([128, 128], BF16, tag="warm")
            nc.vector.memset(wu[:], 1.0)
            wu_ps = ps_sc.tile([128, 1024], F32, tag="sc")
            for _ in range(warmup_mms):
                nc.tensor.matmul(wu_ps[:, 0:128], wu[:], wu[:],
                                 start=True, stop=True,
                                 skip_group_check=True)

            def sctile():
                t = ps_sc.tile([128, 1024], F32, tag="sc")
                return t

            def xslab(g, half):
                xg = xT_g[g]
                if isinstance(xg, tuple):
                    return xg[half].rearrange("p c f -> p (c f)")
                return xg.rearrange("p c f -> p (c f)")[
                    :, half * 512:(half + 1) * 512]

            def xchunk(g, j):
                xg = xT_g[g]
                if isinstance(xg, tuple):
                    return xg[j // 4][:, j % 4, :]
                return xg[:, j, :]

            kT = [None] * 4
            qT = [None] * 4
            v_g = [None] * 4

            # --- projection helpers (psum borrowed from the scores pool) -
            def emit_k_half(g, half):
                if half == 0:
                    tg = big.tile([128, 1024], F32R, tag=f"kT{g}")
                    kT[g] = tg
                else:
                    tg = kT[g]
                pst = sctile()
                lo, hi = half * 512, half * 512 + 512
                nc.tensor.matmul(pst[:, 0:512], wkT, xslab(g, half),
                                 start=True, stop=True)
                nc.scalar.activation(tg[:, lo:hi], pst[:, 0:512], Ident,
                                     bias=bk)

            def v_bias_copy(vg, psv3, lo, hi):
                n = hi - lo
                bvx = bv_bc[:, None, :].to_broadcast((128, n, 128))
                nc.vector.tensor_tensor(
                    vg[:, lo:hi, :], psv3[:, lo:hi, :], bvx, ADD)

            def xbchunk(g, j):
                return xB_g[g][:, j, :]

            def emit_v(g):
                vg = big.tile([128, 8, 128], BF16, tag=f"v{g}")
                psv = sctile()
                psv3 = psv.rearrange("p (c f) -> p c f", f=128)
                for j in range(8):
                    nc.tensor.matmul(
                        psv[:, j * 128:(j + 1) * 128], xbchunk(g, j), wvT_bf,
                        start=True, stop=True,
                    )
                v_bias_copy(vg, psv3, 0, 8)
                v_g[g] = vg

            def q_slice(mc):
                return qT[mc // 8][:, (mc % 8) * 128:(mc % 8 + 1) * 128]

            def v_chunk(mc):
                return v_g[mc // 8][:, mc % 8, :]

            # --- prologue ------------------------------------------------
            # q0[0:512] rushed into the warmup tile's spare columns
            qt0 = big.tile([128, 1024], F32R, tag="qT0")
            qT[0] = qt0
            nc.tensor.matmul(wu_ps[:, 512:640], wqT,
                             xslab(0, 0)[:, 0:128], start=True, stop=True)
            nc.vector.tensor_scalar_add(qt0[:, 0:128], wu_ps[:, 512:640], bq)
            nc.tensor.matmul(wu_ps[:, 640:1024], wqT,
                             xslab(0, 0)[:, 128:512], start=True, stop=True)
            # k0
            kga = big.tile([128, 512], F32R, tag="kT0a")
            kgb = big.tile([128, 512], F32R, tag="kT0b")
            k0ps = sctile()
            nc.tensor.matmul(k0ps[:, 0:512], wkT, xslab(0, 0),
                             start=True, stop=True)
            nc.vector.tensor_scalar_add(kga[:], k0ps[:, 0:512], bk)
            kT[0] = (kga, kgb)
            # scores chunk 0 first half + exp
            q_sl0 = q_slice(0)
            s0 = sctile()
            pT0 = ptp.tile([128, 1024], I16, tag="pt")
            pT0b = pT0.bitcast(BF16)
            nc.tensor.matmul(s0[:, 0:512], q_sl0, kga[:], start=True, stop=True)
            nc.scalar.activation(pT0b[:, 0:512], s0[:, 0:512], Exp)
            # k0 second half + scores 0 second half
            nc.tensor.matmul(k0ps[:, 512:1024], wkT, xslab(0, 1),
                             start=True, stop=True)
            nc.vector.tensor_scalar_add(kgb[:], k0ps[:, 512:1024], bk)
            nc.tensor.matmul(s0[:, 512:1024], q_sl0, kgb[:], start=True, stop=True)
            nc.scalar.activation(pT0b[:, 512:1024], s0[:, 512:1024], Exp)
            # deferred weight casts + v0 + q0 tail
            nc.vector.tensor_copy(wbig[:, 256:384], wf32[:, 256:384])
            nc.vector.tensor_copy(wvT_bf[:], wf32[:, 256:384])
            nc.vector.memset(ones_bf, 1.0)
            orow_f = big.tile([1, 128], F32, tag="orf")
            orow = big.tile([1, 128], F32R, tag="oro")
            nc.vector.memset(orow_f[:], 1.0)
            nc.vector.tensor_copy(orow[:], orow_f[:])
            emit_v(0)
            nc.vector.tensor_scalar_add(qt0[:, 128:512], wu_ps[:, 640:1024], bq)
            q0ps = sctile()
            nc.tensor.matmul(q0ps[:, 512:1024], wqT, xslab(0, 1),
                             start=True, stop=True)
            nc.vector.tensor_scalar_add(qt0[:, 512:1024], q0ps[:, 512:1024], bq)

            fin_jobs = [None]
            setup_sched = {}

            def add_setup(mc, fn):
                setup_sched.setdefault(mc, []).append(fn)

            setup_state = {}

            def emit_q_mm(g):
                tg = big.tile([128, 1024], F32R, tag=f"qT{g}")
                pst = sctile()
                nc.tensor.matmul(pst[:, 0:512], wqT, xslab(g, 0),
                                 start=True, stop=True)
                nc.tensor.matmul(pst[:, 512:1024], wqT, xslab(g, 1),
                                 start=True, stop=True)
                setup_state[f"q{g}"] = (tg, pst)
                qT[g] = tg

            def emit_q_add(g):
                tg, pst = setup_state.pop(f"q{g}")
                nc.scalar.activation(tg[:], pst[:], Ident, bias=bq)

            def emit_k_mm(g):
                tg = big.tile([128, 1024], F32R, tag=f"kT{g}")
                pst = sctile()
                nc.tensor.matmul(pst[:, 0:512], wkT, xslab(g, 0),
                                 start=True, stop=True)
                nc.tensor.matmul(pst[:, 512:1024], wkT, xslab(g, 1),
                                 start=True, stop=True)
                setup_state[f"k{g}"] = (tg, pst)
                kT[g] = tg

            def emit_k_add(g):
                tg, pst = setup_state.pop(f"k{g}")
                nc.scalar.activation(tg[:], pst[:], Ident, bias=bk)

            def emit_v_mm(g, quarter):
                if quarter == 0:
                    vg = big.tile([128, 8, 128], BF16, tag=f"v{g}")
                    psv = sctile()
                    setup_state[f"v{g}"] = (vg, psv)
                    v_g[g] = vg
                else:
                    vg, psv = setup_state[f"v{g}"]
                for j in range(quarter * 2, quarter * 2 + 2):
                    nc.tensor.matmul(
                        psv[:, j * 128:(j + 1) * 128], xbchunk(g, j), wvT_bf,
                        start=True, stop=True,
                    )

            def emit_v_copy(g):
                vg, psv = setup_state.pop(f"v{g}")
                psv3 = psv.rearrange("p (c f) -> p c f", f=128)
                v_bias_copy(vg, psv3, 0, 8)

            add_setup(2, lambda: emit_q_mm(1))
            add_setup(2, lambda: emit_dma_x(2))
            add_setup(4, lambda: emit_q_add(1))
            add_setup(4, lambda: emit_dma_x(3))
            for s, qq in ((4, 0), (5, 1), (6, 2), (7, 3)):
                add_setup(s, lambda q=qq: emit_v_mm(1, q))
            add_setup(8, lambda: emit_v_copy(1))
            add_setup(10, lambda: emit_q_mm(2))
            add_setup(12, lambda: emit_q_add(2))
            for s, qq in ((12, 0), (13, 1), (14, 2), (15, 3)):
                add_setup(s, lambda q=qq: emit_v_mm(2, q))
            add_setup(16, lambda: emit_v_copy(2))
            add_setup(18, lambda: emit_q_mm(3))
            add_setup(20, lambda: emit_q_add(3))
            for s, qq in ((20, 0), (21, 1), (22, 2), (23, 3)):
                add_setup(s, lambda q=qq: emit_v_mm(3, q))
            add_setup(24, lambda: emit_v_copy(3))
            add_setup(26, lambda: emit_k_mm(1))
            add_setup(28, lambda: emit_k_add(1))

            pT_prev = pT0
            last_pv_chain = [None]

            for sec in range(NSEC):
                d_acc_a = wrk.tile([128, dcol], BF16, tag="dea")
                d_acc_b = wrk.tile([128, dcol], BF16, tag="deb")
                d_acc = [d_acc_a, d_acc_b]
                d_odd_a = wrk.tile([128, ecol], BF16, tag="doa")
                d_odd_b = wrk.tile([128, ecol], BF16, tag="dob")
                d_odd = [d_odd_a, d_odd_b]
                ps_pv_t = ps_acc.tile([128, 1024], F32, tag="acct")

                def emit_scores(mc, sec=sec):
                    ps_s = sctile()
                    q_sl = q_slice(mc)
                    kg = kT[sec]
                    if isinstance(kg, tuple):
                        ka, kb = kg[0][:], kg[1][:]
                    else:
                        ka, kb = kg[:, 0:512], kg[:, 512:1024]
                    nc.tensor.matmul(ps_s[:, 0:512], q_sl, ka,
                                     start=True, stop=True)
                    nc.tensor.matmul(ps_s[:, 512:1024], q_sl, kb,
                                     start=True, stop=True)
                    return ps_s

                def emit_exp(ps_s, mc):
                    # one reader per PSUM scores tile: whole-chunk exp on
                    # DVE (Schraudolph, mc%4==1) or ACT (exact), a single
                    # instruction either way
                    pT = ptp.tile([128, 1024], I16, tag="pt")
                    if mc % 4 == 1:
                        nc.vector.tensor_scalar(
                            pT[:], ps_s[:], SCH_A, SCH_B, MULT, ADD)
                    else:
                        nc.scalar.activation(pT.bitcast(BF16)[:], ps_s[:], Exp)
                    return pT

                def emit_pv(mc, pT, ps_pv_t=ps_pv_t):
                    pTb = pT.bitcast(BF16)
                    nc.tensor.matmul(
                        ps_pv_t[:, 0:512], v_chunk(mc), pTb[:, 0:512],
                        start=(mc == 0), stop=(mc == NC - 1),
                        skip_group_check=True,
                    )
                    nc.tensor.matmul(
                        ps_pv_t[:, 512:1024], v_chunk(mc), pTb[:, 512:1024],
                        start=(mc == 0), stop=(mc == NC - 1),
                        skip_group_check=True,
                    )

                def emit_denom(mc, pT, d_odd=d_odd, d_acc=d_acc, sec=sec):
                    pTb = pT.bitcast(BF16)
                    da = d_acc[mc % 2]
                    if mc < 2:
                        nc.vector.tensor_copy(da[:], pTb[:, 0:dcol])
                    else:
                        nc.vector.tensor_tensor(
                            da[:], da[:], pTb[:, 0:dcol], ADD)
                    do = d_odd[mc % 2]
                    if mc < 2:
                        nc.gpsimd.tensor_copy(do[:], pTb[:, dcol:SEC])
                    else:
                        nc.gpsimd.tensor_tensor(
                            do[:], do[:], pTb[:, dcol:SEC], ADD)

                # finalize of THIS section, spread over the next section:
                # og copy-out (frees acc) -> denom row (ones-matmuls into a
                # borrowed scores tile) -> reciprocal row -> gpsimd
                # partition broadcast -> og *= rdbc -> straight DMA out
                def make_fin(sec=sec, ps_pv_t=ps_pv_t,
                             d_odd=d_odd, d_acc=d_acc, pT31=None):
                    st = {}
                    last = (sec == NSEC - 1)

                    def f_ocopy(half):
                        if "og" not in st:
                            og = big.tile([128, 1024], F32,
                                          tag=f"out{sec % 2}")
                            st["og"] = og
                        lo, hi = half * 512, half * 512 + 512
                        if half == 0:
                            nc.scalar.copy(st["og"][:, lo:hi],
                                           ps_pv_t[:, lo:hi])
                        else:
                            nc.vector.tensor_copy(st["og"][:, lo:hi],
                                                  ps_pv_t[:, lo:hi])

                    def f_rowmm():
                        ftr = sctile()
                        st["ftr"] = ftr
                        row = ftr[0:1, :]
                        for i in range(2):
                            nc.tensor.matmul(
                                row[:, 0:512], ones_bf, d_acc[i][:, 0:512],
                                start=(i == 0), stop=(i == 1),
                                skip_group_check=True)
                        for i in range(2):
                            nc.tensor.matmul(
                                row[:, 512:dcol], ones_bf,
                                d_acc[i][:, 512:dcol],
                                start=(i == 0), stop=(i == 1),
                                skip_group_check=True)
                        for i in range(2):
                            nc.tensor.matmul(
                                row[:, dcol:SEC], ones_bf, d_odd[i][:],
                                start=(i == 0), stop=(i == 1),
                                skip_group_check=True)

                    def f_recip():
                        rd = wrk.tile([1, 1024], F32, tag="rdr")
                        st["rd"] = rd
                        nc.vector.reciprocal(rd[:], st["ftr"][0:1, :])

                    def f_bc():
                        rdbc = wrk.tile([128, 1024], F32, tag="rdbc")
                        st["rdbc"] = rdbc
                        nc.gpsimd.partition_broadcast(rdbc[:], st["rd"][:])

                    def mk_scale(half):
                        def f_scale():
                            lo, hi = half * 512, half * 512 + 512
                            nc.vector.tensor_tensor(
                                st["og"][:, lo:hi], st["og"][:, lo:hi],
                                st["rdbc"][:, lo:hi], MULT)
                        return f_scale

                    def mk_dma(half, eng):
                        def f_dma():
                            lo, hi = half * 512, half * 512 + 512
                            eng().dma_start(
                                y[:, sec * SEC + lo:sec * SEC + hi],
                                st["og"][:, lo:hi],
                            )
                        return f_dma

                    def f_rowmm_h(half):
                        if half == 0:
                            ftr = sctile()
                            st["ftr"] = ftr
                        row = st["ftr"][0:1, :]
                        p31 = pT31.bitcast(BF16)
                        if half == 0:
                            for i in range(2):
                                nc.tensor.matmul(
                                    row[:, 0:512], ones_bf,
                                    d_acc[i][:, 0:512],
                                    start=(i == 0), stop=False,
                                    skip_group_check=True)
                            nc.tensor.matmul(
                                row[:, 0:512], ones_bf, p31[:, 0:512],
                                start=False, stop=True,
                                skip_group_check=True)
                        else:
                            for i in range(2):
                                nc.tensor.matmul(
                                    row[:, 512:dcol], ones_bf,
                                    d_acc[i][:, 512:dcol],
                                    start=(i == 0), stop=False,
                                    skip_group_check=True)
                            nc.tensor.matmul(
                                row[:, 512:dcol], ones_bf, p31[:, 512:dcol],
                                start=False, stop=True,
                                skip_group_check=True)
                            for i in range(2):
                                nc.tensor.matmul(
                                    row[:, dcol:SEC], ones_bf, d_odd[i][:],
                                    start=(i == 0), stop=False,
                                    skip_group_check=True)
                            nc.tensor.matmul(
                                row[:, dcol:SEC], ones_bf, p31[:, dcol:SEC],
                                start=False, stop=True,
                                skip_group_check=True)

                    def f_recip_h(half):
                        if half == 0:
                            rd = wrk.tile([1, 1024], F32R, tag="rdr2")
                            st["rd"] = rd
                        lo, hi = half * 512, half * 512 + 512
                        with nc.allow_low_precision(
                                reason="f32r reciprocal row for PE bc"):
                            nc.vector.reciprocal(st["rd"][:, lo:hi],
                                                 st["ftr"][0:1, lo:hi])

                    def f_bc_h(half):
                        if half == 0:
                            rdbc = sctile()
                            st["rdbc"] = rdbc
                        lo, hi = half * 512, half * 512 + 512
                        nc.tensor.matmul(
                            st["rdbc"][:, lo:hi], orow, st["rd"][:, lo:hi],
                            start=True, stop=True, skip_group_check=True)

                    if not last:
                        steps = [lambda: f_ocopy(0), lambda: f_ocopy(1),
                                 f_rowmm, f_recip, f_bc,
                                 mk_scale(0), mk_dma(0, lambda: nc.sync),
                                 mk_scale(1), mk_dma(1, lambda: nc.scalar)]
                        return steps
                    def mk_scale_q(q):
                        def f_s():
                            lo, hi = q * 256, q * 256 + 256
                            nc.vector.tensor_tensor(
                                st["og"][:, lo:hi], st["og"][:, lo:hi],
                                st["rdbc"][:, lo:hi], MULT)
                        return f_s

                    def mk_dma_q(q, eng):
                        def f_dma():
                            lo, hi = q * 256, q * 256 + 256
                            eng().dma_start(
                                y[:, sec * SEC + lo:sec * SEC + hi],
                                st["og"][:, lo:hi],
                            )
                        return f_dma

                    # last section: pipelined halves; the second half's
                    # scale+DMA go out in quarters on alternating queues so
                    # the very last transfer is small
                    steps = [lambda: f_ocopy(0),
                             lambda: f_rowmm_h(0), lambda: f_recip_h(0),
                             lambda: f_bc_h(0),
                             lambda: f_ocopy(1),
                             mk_scale(0), mk_dma(0, lambda: nc.sync),
                             lambda: f_rowmm_h(1), lambda: f_recip_h(1),
                             lambda: f_bc_h(1),
                             mk_scale_q(2), mk_dma_q(2, lambda: nc.scalar),
                             mk_scale_q(3), mk_dma_q(3, lambda: nc.sync)]
                    return steps

                fin_positions = [2, 3, 4, 5, 6, 7, 8, 9, 10]

                pT_hist = {}
                if sec == 0:
                    pT_hist[0] = pT_prev
                for mc in range(NC):
                    if not (mc == 0 and sec == 0):
                        ps_s = emit_scores(mc)
                    if mc < PVLAG and last_pv_chain[0]:
                        last_pv_chain[0][mc]()
                    if mc >= PVLAG:
                        emit_pv(mc - PVLAG, pT_hist.pop(mc - PVLAG))
                    if mc >= 1:
                        emit_denom(mc - 1, pT_hist[mc - 1])
                    if not (mc == 0 and sec == 0):
                        pT_hist[mc] = emit_exp(ps_s, mc)
                    if mc >= 1:
                        if sec == 0 and mc in setup_sched:
                            for fn in setup_sched[mc]:
                                fn()
                        if sec > 0 and fin_jobs[0]:
                            if mc in fin_positions:
                                idx = fin_positions.index(mc)
                                if idx < len(fin_jobs[0]):
                                    fin_jobs[0][idx]()
                        if sec == 1 and mc == 18:
                            emit_k_half(2, 0)
                        elif sec == 1 and mc == 22:
                            emit_k_half(2, 1)
                        elif sec == 2 and mc == 18:
                            emit_k_half(3, 0)
                        elif sec == 2 and mc == 22:
                            emit_k_half(3, 1)

                # pend the last PVLAG PVs + last denom into the next section
                def mk_last(ps=[pT_hist[NC - PVLAG + i] for i in range(PVLAG)],
                            pv=emit_pv, dn=emit_denom, sec=sec):
                    jobs = []
                    for i in range(PVLAG):
                        def run(i=i):
                            pv(NC - PVLAG + i, ps[i])
                            if i == PVLAG - 1 and sec != NSEC - 1:
                                dn(NC - 1, ps[i])
                        jobs.append(run)
                    return jobs

                last_pv_chain[0] = mk_last()
                fin_jobs[0] = make_fin(pT31=pT_hist[NC - 1])

            # drain: last section's PVs + finalize immediately
            if last_pv_chain[0] is not None:
                for fn in last_pv_chain[0]:
                    fn()
                last_pv_chain[0] = None
            for step in fin_jobs[0]:
                step()

    nc.finalize()
    return nc


def _get_nc():
    if "nc" not in _cache:
        _cache["nc"] = _build()
    return _cache["nc"]


def make_wp(Wk, Wq, Wv, bk, bq, bv):
    # layout: startup-critical wk/wq first so their DMA can land alone
    wp = np.zeros((128, 643), np.float32)
    wp[:, 0:128] = Wk.T
    wp[:, 128:256] = Wq.T
    wp[:, 256:384] = Wv.T
    wp[:, 384:512] = np.eye(128, dtype=np.float32)
    wp[:, 512] = 1.0
    wp[:, 513:641] = np.broadcast_to(bv[None, :], (128, 128))
    wp[:, 641] = bk
    wp[:, 642] = bq
    return wp


def kernel(x, Wk, bk, Wq, bq, Wv, bv, **_ignored):
    from concourse.bass_utils import run_bass_kernel_spmd

    x = np.asarray(x, dtype=np.float32)
    wp = make_wp(
        np.asarray(Wk, np.float32), np.asarray(Wq, np.float32),
        np.asarray(Wv, np.float32), np.asarray(bk, np.float32),
        np.asarray(bq, np.float32), np.asarray(bv, np.float32),
    )

    nc = _get_nc()
    import ml_dtypes
    in_maps = [
        {"xt": np.ascontiguousarray(x[b].T),
         "xb": np.ascontiguousarray(x[b].T).astype(ml_dtypes.bfloat16),
         "wp": wp} for b in range(B)
    ]
    res = run_bass_kernel_spmd(nc, in_maps, core_ids=list(range(B)))
    out = np.stack(
        [np.ascontiguousarray(res.results[b]["y"].T) for b in range(B)],
        axis=0)
    return out


# revision 40
# speedup vs baseline: 1.0070x; 1.0047x over previous
"""Fake-attention kernel for trn2: 8 NeuronCores, one batch element per core.

Per core (batch b): out = softmax(k @ q^T) @ v, with k/q/v = x @ W.T + b.

Dataflow (transposed so the PV contraction lands on partitions):
  xT [f,n]     <- transposed on host; loaded via gpsimd casting DMAs so
                  the tiles carry fp32r (full-rate PE) directly
  kT,qT [d,n]  = W @ xT (fp32r) + bias at the PSUM->SBUF copy-out
                 (setup adds on ACT, prologue on DVE)
  v [m,d]      = xb-chunks (host-prerounded bf16 x) as lhsT, rhs = Wv^T
                 in bf16 (full PE rate at 128 cols); the copy-out adds bv
                 broadcast along d and writes BF16 (bv rides the PV sum
                 exactly because softmax weights sum to 1)
  per n-section of 1024, m-chunks of 128 (PV lags scores by 3 chunks;
  the 3-deep PSUM scores pool means the exp of chunk c never gates the
  scores matmul of chunk c+2 — the exp->scores chain that capped the
  2-buffer design is gone):
    scoresT chunk [m=128, n=1024] = qT-slice as lhsT, kT as rhs (fp32r)
    pT = exp(scoresT): chunks mc%4==1 on DVE as a Schraudolph bf16 bit
    pattern (single fused mult+add, int16 convert-on-write), the rest on
    ACT (PSUM->SBUF, bf16 convert-on-write); each scores tile has exactly
    one reader (PSUM tiles serialize readers)
    outT [d,n] += v-chunk(bf16) as lhsT, pT(bf16) as rhs (PSUM acc, one
    shared accumulator buffer - the copy-out below frees it in time)
    denominator: column split - DVE adds cols [0:dcol] in BF16 (2x_1p,
    two ping-pong accumulators for precision), GPSIMD adds cols
    [dcol:1024] (also BF16 pairs; GPSIMD cost is dtype-blind)
  finalize (spread over the NEXT section's chunk stream):
    og = PSUM outT -> SBUF f32 (ACT half / DVE half; frees the acc buf)
    denom row [1,1024] via ones-matmuls from the bf16 accumulators into a
    borrowed scores tile -> DVE reciprocal row -> GPSIMD
    partition_broadcast -> og *= rdbc (two DVE sbuf tensor_tensor mults)
    DMA halves straight out in [d, n] orientation over the SP/ACT queues
    (y is declared transposed [D, N]; the host transposes back).

PSUM (8 banks): scores pool 3x[128,1024]f32 (6 banks) + acc 1x (2).
Setup projections (k/q/v) borrow scores-pool tiles; the denominator row
borrows one for two steps of the finalize.
"""
import numpy as np

B = 8
N = 4096
D = 128
NC = 32          # chunks of 128 along n/m
NSEC = 4         # sections of 1024 along n
SEC = 1024
DCOL = 704       # denominator cols on DVE (d_acc); rest on GPSIMD (d_odd)
SCH_A = 184.6646884   # 2^7 * log2(e): Schraudolph exp in bf16 bit space
SCH_B = 16248.0       # (127 - c) * 2^7, calibrated on the real score range

_cache = {}


def _build(dcol=DCOL, warmup_mms=16, ptp_bufs=7):
    import concourse.bass as bass  # noqa
    import concourse.mybir as mybir
    import concourse.tile as tile
    from concourse import bacc

    F32 = mybir.dt.float32
    F32R = mybir.dt.float32r
    BF16 = mybir.dt.bfloat16
    I16 = mybir.dt.int16
    Exp = mybir.ActivationFunctionType.Exp
    Ident = mybir.ActivationFunctionType.Identity
    ADD = mybir.AluOpType.add
    MULT = mybir.AluOpType.mult
    ecol = SEC - dcol
    PVLAG = 3

    nc = bacc.Bacc()
    xt = nc.declare_dram_parameter("xt", [D, N], F32, isOutput=False)
    xb = nc.declare_dram_parameter("xb", [D, N], BF16, isOutput=False)
    wp = nc.declare_dram_parameter("wp", [128, 643], F32, isOutput=False)
    y = nc.declare_dram_parameter("y", [D, N], F32, isOutput=True)

    xt_dram = xt.rearrange("p (c l) -> p c l", l=128)
    xb_dram = xb.rearrange("p (c l) -> p c l", l=128)

    with tile.TileContext(nc) as tc:
        with (
            tc.tile_pool(name="big", bufs=1) as big,
            tc.tile_pool(name="ptp", bufs=ptp_bufs) as ptp,
            tc.tile_pool(name="wrk", bufs=2) as wrk,
            tc.tile_pool(name="sc", bufs=3, space="PSUM") as ps_sc,
            tc.tile_pool(name="acc", bufs=1, space="PSUM") as ps_acc,
        ):
            # --- input DMAs, criticality-ordered -------------------------
            xg0a = big.tile([128, 4, 128], F32R, tag="xT0a")
            xg0b = big.tile([128, 4, 128], F32R, tag="xT0b")
            wf32 = big.tile([128, 512], F32, tag="wf32")
            wbig = big.tile([128, 512], F32R, tag="wbig")
            wrest = big.tile([128, 131], F32, tag="wrest")
            wsm = big.tile([128, 130], BF16, tag="wsm")
            nc.gpsimd.dma_start(xg0a[:], xt_dram[:, 0:4, :])
            nc.gpsimd.dma_start(xg0b[:], xt_dram[:, 4:8, :])
            nc.sync.dma_start(wf32[:, 0:256], wp[:, 0:256])
            nc.sync.dma_start(wrest[:], wp[:, 512:643])
            nc.sync.dma_start(wf32[:, 256:512], wp[:, 256:512])
            # rounding copies: critical wq/wk now; wv after the prologue's
            # k-chain (only needed for v0)
            nc.vector.tensor_copy(wbig[:, 128:256], wf32[:, 128:256])
            nc.vector.tensor_copy(wbig[:, 0:128], wf32[:, 0:128])
            wkT = wbig[:, 0:128]
            wqT = wbig[:, 128:256]
            wvT = wbig[:, 256:384]
            ones_bf = wsm[:, 0:1]
            wvT_bf = wsm[:, 2:130]
            bv_bc = wrest[:, 1:129]
            bk = wrest[:, 129:130]
            bq = wrest[:, 130:131]

            xT_g = [None] * 4
            xT_g[0] = (xg0a, xg0b)
            xB_g = [None] * 4
            xb0 = big.tile([128, 8, 128], BF16, tag="xB0")
            nc.sync.dma_start(xb0[:], xb_dram[:, 0:8, :])
            xB_g[0] = xb0

            def emit_dma_x(g):
                xg = big.tile([128, 8, 128], F32R, tag=f"xT{g}")
                nc.gpsimd.dma_start(xg[:], xt_dram[:, g * 8:(g + 1) * 8, :])
                xT_g[g] = xg
                xbg = big.tile([128, 8, 128], BF16, tag=f"xB{g}")
                nc.sync.dma_start(xbg[:], xb_dram[:, g * 8:(g + 1) * 8, :])
                xB_g[g] = xbg

            emit_dma_x(1)

            wu = big.tile([128, 128], BF16, tag="warm")
            nc.vector.memset(wu[:], 1.0)
            wu_ps = ps_sc.tile([128, 1024], F32, tag="sc")
            for _ in range(warmup_mms):
                nc.tensor.matmul(wu_ps[:, 0:128], wu[:], wu[:],
                                 start=True, stop=True,
                                 skip_group_check=True)

            def sctile():
                t = ps_sc.tile([128, 1024], F32, tag="sc")
                return t

            def xslab(g, half):
                xg = xT_g[g]
                if isinstance(xg, tuple):
                    return xg[half].rearrange("p c f -> p (c f)")
                return xg.rearrange("p c f -> p (c f)")[
                    :, half * 512:(half + 1) * 512]

            def xchunk(g, j):
                xg = xT_g[g]
                if isinstance(xg, tuple):
                    return xg[j // 4][:, j % 4, :]
                return xg[:, j, :]

            kT = [None] * 4
            qT = [None] * 4
            v_g = [None] * 4

            # --- projection helpers (psum borrowed from the scores pool) -
            def emit_k_half(g, half):
                if half == 0:
                    tg = big.tile([128, 1024], F32R, tag=f"kT{g}")
                    kT[g] = tg
                else:
                    tg = kT[g]
                pst = sctile()
                lo, hi = half * 512, half * 512 + 512
                nc.tensor.matmul(pst[:, 0:512], wkT, xslab(g, half),
                                 start=True, stop=True)
                nc.scalar.activation(tg[:, lo:hi], pst[:, 0:512], Ident,
                                     bias=bk)

            def v_bias_copy(vg, psv3, lo, hi):
                n = hi - lo
                bvx = bv_bc[:, None, :].to_broadcast((128, n, 128))
                nc.vector.tensor_tensor(
                    vg[:, lo:hi, :], psv3[:, lo:hi, :], bvx, ADD)

            def xbchunk(g, j):
                return xB_g[g][:, j, :]

            def emit_v(g):
                vg = big.tile([128, 8, 128], BF16, tag=f"v{g}")
                psv = sctile()
                psv3 = psv.rearrange("p (c f) -> p c f", f=128)
                for j in range(8):
                    nc.tensor.matmul(
                        psv[:, j * 128:(j + 1) * 128], xbchunk(g, j), wvT_bf,
                        start=True, stop=True,
                    )
                v_bias_copy(vg, psv3, 0, 8)
                v_g[g] = vg

            def q_slice(mc):
                return qT[mc // 8][:, (mc % 8) * 128:(mc % 8 + 1) * 128]

            def v_chunk(mc):
                return v_g[mc // 8][:, mc % 8, :]

            # --- prologue ------------------------------------------------
            # q0[0:512] rushed into the warmup tile's spare columns
            qt0 = big.tile([128, 1024], F32R, tag="qT0")
            qT[0] = qt0
            nc.tensor.matmul(wu_ps[:, 512:640], wqT,
                             xslab(0, 0)[:, 0:128], start=True, stop=True)
            nc.vector.tensor_scalar_add(qt0[:, 0:128], wu_ps[:, 512:640], bq)
            nc.tensor.matmul(wu_ps[:, 640:1024], wqT,
                             xslab(0, 0)[:, 128:512], start=True, stop=True)
            # k0
            kga = big.tile([128, 512], F32R, tag="kT0a")
            kgb = big.tile([128, 512], F32R, tag="kT0b")
            k0ps = sctile()
            nc.tensor.matmul(k0ps[:, 0:512], wkT, xslab(0, 0),
                             start=True, stop=True)
            nc.vector.tensor_scalar_add(kga[:], k0ps[:, 0:512], bk)
            kT[0] = (kga, kgb)
            # scores chunk 0 first half + exp
            q_sl0 = q_slice(0)
            s0 = sctile()
            pT0 = ptp.tile([128, 1024], I16, tag="pt")
            pT0b = pT0.bitcast(BF16)
            nc.tensor.matmul(s0[:, 0:512], q_sl0, kga[:], start=True, stop=True)
            nc.scalar.activation(pT0b[:, 0:512], s0[:, 0:512], Exp)
            # k0 second half + scores 0 second half
            nc.tensor.matmul(k0ps[:, 512:1024], wkT, xslab(0, 1),
                             start=True, stop=True)
            nc.vector.tensor_scalar_add(kgb[:], k0ps[:, 512:1024], bk)
            nc.tensor.matmul(s0[:, 512:1024], q_sl0, kgb[:], start=True, stop=True)
            nc.scalar.activation(pT0b[:, 512:1024], s0[:, 512:1024], Exp)
            # deferred weight casts + v0 + q0 tail
            nc.vector.tensor_copy(wbig[:, 256:384], wf32[:, 256:384])
            nc.vector.tensor_copy(wvT_bf[:], wf32[:, 256:384])
            nc.vector.memset(ones_bf, 1.0)
            orow_f = big.tile([1, 128], F32, tag="orf")
            orow = big.tile([1, 128], F32R, tag="oro")
            nc.vector.memset(orow_f[:], 1.0)
            nc.vector.tensor_copy(orow[:], orow_f[:])
            emit_v(0)
            nc.vector.tensor_scalar_add(qt0[:, 128:512], wu_ps[:, 640:1024], bq)
            q0ps = sctile()
            nc.tensor.matmul(q0ps[:, 512:1024], wqT, xslab(0, 1),
                             start=True, stop=True)
            nc.vector.tensor_scalar_add(qt0[:, 512:1024], q0ps[:, 512:1024], bq)

            fin_jobs = [None]
            setup_sched = {}

            def add_setup(mc, fn):
                setup_sched.setdefault(mc, []).append(fn)

            setup_state = {}

            def emit_q_mm(g):
                tg = big.tile([128, 1024], F32R, tag=f"qT{g}")
                pst = sctile()
                nc.tensor.matmul(pst[:, 0:512], wqT, xslab(g, 0),
                                 start=True, stop=True)
                nc.tensor.matmul(pst[:, 512:1024], wqT, xslab(g, 1),
                                 start=True, stop=True)
                setup_state[f"q{g}"] = (tg, pst)
                qT[g] = tg

            def emit_q_add(g):
                tg, pst = setup_state.pop(f"q{g}")
                nc.scalar.activation(tg[:], pst[:], Ident, bias=bq)

            def emit_k_mm(g):
                tg = big.tile([128, 1024], F32R, tag=f"kT{g}")
                pst = sctile()
                nc.tensor.matmul(pst[:, 0:512], wkT, xslab(g, 0),
                                 start=True, stop=True)
                nc.tensor.matmul(pst[:, 512:1024], wkT, xslab(g, 1),
                                 start=True, stop=True)
                setup_state[f"k{g}"] = (tg, pst)
                kT[g] = tg

            def emit_k_add(g):
                tg, pst = setup_state.pop(f"k{g}")
                nc.scalar.activation(tg[:], pst[:], Ident, bias=bk)

            def emit_v_mm(g, quarter):
                if quarter == 0:
                    vg = big.tile([128, 8, 128], BF16, tag=f"v{g}")
                    psv = sctile()
                    setup_state[f"v{g}"] = (vg, psv)
                    v_g[g] = vg
                else:
                    vg, psv = setup_state[f"v{g}"]
                for j in range(quarter * 2, quarter * 2 + 2):
                    nc.tensor.matmul(
                        psv[:, j * 128:(j + 1) * 128], xbchunk(g, j), wvT_bf,
                        start=True, stop=True,
                    )

            def emit_v_copy(g):
                vg, psv = setup_state.pop(f"v{g}")
                psv3 = psv.rearrange("p (c f) -> p c f", f=128)
                v_bias_copy(vg, psv3, 0, 8)

            add_setup(2, lambda: emit_q_mm(1))
            add_setup(2, lambda: emit_dma_x(2))
            add_setup(4, lambda: emit_q_add(1))
            add_setup(4, lambda: emit_dma_x(3))
            for s, qq in ((4, 0), (5, 1), (6, 2), (7, 3)):
                add_setup(s, lambda q=qq: emit_v_mm(1, q))
            add_setup(8, lambda: emit_v_copy(1))
            add_setup(10, lambda: emit_q_mm(2))
            add_setup(12, lambda: emit_q_add(2))
            for s, qq in ((12, 0), (13, 1), (14, 2), (15, 3)):
                add_setup(s, lambda q=qq: emit_v_mm(2, q))
            add_setup(16, lambda: emit_v_copy(2))
            add_setup(18, lambda: emit_q_mm(3))
            add_setup(20, lambda: emit_q_add(3))
            for s, qq in ((20, 0), (21, 1), (22, 2), (23, 3)):
                add_setup(s, lambda q=qq: emit_v_mm(3, q))
            add_setup(24, lambda: emit_v_copy(3))
            add_setup(26, lambda: emit_k_mm(1))
            add_setup(28, lambda: emit_k_add(1))

            pT_prev = pT0
            last_pv_chain = [None]

            for sec in range(NSEC):
                d_acc_a = wrk.tile([128, dcol], BF16, tag="dea")
                d_acc_b = wrk.tile([128, dcol], BF16, tag="deb")
                d_acc = [d_acc_a, d_acc_b]
                d_odd_a = wrk.tile([128, ecol], BF16, tag="doa")
                d_odd_b = wrk.tile([128, ecol], BF16, tag="dob")
                d_odd = [d_odd_a, d_odd_b]
                ps_pv_t = ps_acc.tile([128, 1024], F32, tag="acct")

                def emit_scores(mc, sec=sec):
                    ps_s = sctile()
                    q_sl = q_slice(mc)
                    kg = kT[sec]
                    if isinstance(kg, tuple):
                        ka, kb = kg[0][:], kg[1][:]
                    else:
                        ka, kb = kg[:, 0:512], kg[:, 512:1024]
                    nc.tensor.matmul(ps_s[:, 0:512], q_sl, ka,
                                     start=True, stop=True)
                    nc.tensor.matmul(ps_s[:, 512:1024], q_sl, kb,
                                     start=True, stop=True)
                    return ps_s

                def emit_exp(ps_s, mc):
                    # one reader per PSUM scores tile: whole-chunk exp on
                    # DVE (Schraudolph, mc%4==1) or ACT (exact), a single
                    # instruction either way
                    pT = ptp.tile([128, 1024], I16, tag="pt")
                    if mc % 4 == 1:
                        nc.vector.tensor_scalar(
                            pT[:], ps_s[:], SCH_A, SCH_B, MULT, ADD)
                    else:
                        nc.scalar.activation(pT.bitcast(BF16)[:], ps_s[:], Exp)
                    return pT

                def emit_pv(mc, pT, ps_pv_t=ps_pv_t):
                    pTb = pT.bitcast(BF16)
                    nc.tensor.matmul(
                        ps_pv_t[:, 0:512], v_chunk(mc), pTb[:, 0:512],
                        start=(mc == 0), stop=(mc == NC - 1),
                        skip_group_check=True,
                    )
                    nc.tensor.matmul(
                        ps_pv_t[:, 512:1024], v_chunk(mc), pTb[:, 512:1024],
                        start=(mc == 0), stop=(mc == NC - 1),
                        skip_group_check=True,
                    )

                def emit_denom(mc, pT, d_odd=d_odd, d_acc=d_acc, sec=sec):
                    pTb = pT.bitcast(BF16)
                    da = d_acc[mc % 2]
                    if mc < 2:
                        nc.vector.tensor_copy(da[:], pTb[:, 0:dcol])
                    else:
                        nc.vector.tensor_tensor(
                            da[:], da[:], pTb[:, 0:dcol], ADD)
                    do = d_odd[mc % 2]
                    if mc < 2:
                        nc.gpsimd.tensor_copy(do[:], pTb[:, dcol:SEC])
                    else:
                        nc.gpsimd.tensor_tensor(
                            do[:], do[:], pTb[:, dcol:SEC], ADD)

                # finalize of THIS section, spread over the next section:
                # og copy-out (frees acc) -> denom row (ones-matmuls into a
                # borrowed scores tile) -> reciprocal row -> gpsimd
                # partition broadcast -> og *= rdbc -> straight DMA out
                def make_fin(sec=sec, ps_pv_t=ps_pv_t,
                             d_odd=d_odd, d_acc=d_acc, pT31=None):
                    st = {}
                    last = (sec == NSEC - 1)

                    def f_ocopy(half):
                        if "og" not in st:
                            og = big.tile([128, 1024], F32,
                                          tag=f"out{sec % 2}")
                            st["og"] = og
                        lo, hi = half * 512, half * 512 + 512
                        if half == 0:
                            nc.scalar.copy(st["og"][:, lo:hi],
                                           ps_pv_t[:, lo:hi])
                        else:
                            nc.vector.tensor_copy(st["og"][:, lo:hi],
                                                  ps_pv_t[:, lo:hi])

                    def f_rowmm():
                        ftr = sctile()
                        st["ftr"] = ftr
                        row = ftr[0:1, :]
                        for i in range(2):
                            nc.tensor.matmul(
                                row[:, 0:512], ones_bf, d_acc[i][:, 0:512],
                                start=(i == 0), stop=(i == 1),
                                skip_group_check=True)
                        for i in range(2):
                            nc.tensor.matmul(
                                row[:, 512:dcol], ones_bf,
                                d_acc[i][:, 512:dcol],
                                start=(i == 0), stop=(i == 1),
                                skip_group_check=True)
                        for i in range(2):
                            nc.tensor.matmul(
                                row[:, dcol:SEC], ones_bf, d_odd[i][:],
                                start=(i == 0), stop=(i == 1),
                                skip_group_check=True)

                    def f_recip():
                        rd = wrk.tile([1, 1024], F32, tag="rdr")
                        st["rd"] = rd
                        nc.vector.reciprocal(rd[:], st["ftr"][0:1, :])

                    def f_bc():
                        rdbc = wrk.tile([128, 1024], F32, tag="rdbc")
                        st["rdbc"] = rdbc
                        nc.gpsimd.partition_broadcast(rdbc[:], st["rd"][:])

                    def mk_scale(half):
                        def f_scale():
                            lo, hi = half * 512, half * 512 + 512
                            nc.vector.tensor_tensor(
                                st["og"][:, lo:hi], st["og"][:, lo:hi],
                                st["rdbc"][:, lo:hi], MULT)
                        return f_scale

                    def mk_dma(half, eng):
                        def f_dma():
                            lo, hi = half * 512, half * 512 + 512
                            eng().dma_start(
                                y[:, sec * SEC + lo:sec * SEC + hi],
                                st["og"][:, lo:hi],
                            )
                        return f_dma

                    def f_rowmm_h(half):
                        if half == 0:
                            ftr = sctile()
                            st["ftr"] = ftr
                        row = st["ftr"][0:1, :]
                        p31 = pT31.bitcast(BF16)
                        if half == 0:
                            for i in range(2):
                                nc.tensor.matmul(
                                    row[:, 0:512], ones_bf,
                                    d_acc[i][:, 0:512],
                                    start=(i == 0), stop=False,
                                    skip_group_check=True)
                            nc.tensor.matmul(
                                row[:, 0:512], ones_bf, p31[:, 0:512],
                                start=False, stop=True,
                                skip_group_check=True)
                        else:
                            for i in range(2):
                                nc.tensor.matmul(
                                    row[:, 512:dcol], ones_bf,
                                    d_acc[i][:, 512:dcol],
                                    start=(i == 0), stop=False,
                                    skip_group_check=True)
                            nc.tensor.matmul(
                                row[:, 512:dcol], ones_bf, p31[:, 512:dcol],
                                start=False, stop=True,
                                skip_group_check=True)
                            for i in range(2):
                                nc.tensor.matmul(
                                    row[:, dcol:SEC], ones_bf, d_odd[i][:],
                                    start=(i == 0), stop=False,
                                    skip_group_check=True)
                            nc.tensor.matmul(
                                row[:, dcol:SEC], ones_bf, p31[:, dcol:SEC],
                                start=False, stop=True,
                                skip_group_check=True)

                    def f_recip_h(half):
                        if half == 0:
                            rd = wrk.tile([1, 1024], F32R, tag="rdr2")
                            st["rd"] = rd
                        lo, hi = half * 512, half * 512 + 512
                        with nc.allow_low_precision(
                                reason="f32r reciprocal row for PE bc"):
                            nc.vector.reciprocal(st["rd"][:, lo:hi],
                                                 st["ftr"][0:1, lo:hi])

                    def f_bc_h(half):
                        if half == 0:
                            rdbc = sctile()
                            st["rdbc"] = rdbc
                        lo, hi = half * 512, half * 512 + 512
                        nc.tensor.matmul(
                            st["rdbc"][:, lo:hi], orow, st["rd"][:, lo:hi],
                            start=True, stop=True, skip_group_check=True)

                    if not last:
                        steps = [lambda: f_ocopy(0), lambda: f_ocopy(1),
                                 f_rowmm, f_recip, f_bc,
                                 mk_scale(0), mk_dma(0, lambda: nc.sync),
                                 mk_scale(1), mk_dma(1, lambda: nc.scalar)]
                        return steps
                    def mk_scale_q(q):
                        def f_s():
                            lo, hi = q * 256, q * 256 + 256
                            nc.vector.tensor_tensor(
                                st["og"][:, lo:hi], st["og"][:, lo:hi],
                                st["rdbc"][:, lo:hi], MULT)
                        return f_s

                    def mk_dma_q(q, eng):
                        def f_dma():
                            lo, hi = q * 256, q * 256 + 256
                            eng().dma_start(
                                y[:, sec * SEC + lo:sec * SEC + hi],
                                st["og"][:, lo:hi],
                            )
                        return f_dma

                    # last section: pipelined halves; the second half's
                    # scale+DMA go out in quarters on alternating queues so
                    # the very last transfer is small
                    def mk_ocopy_q(q, act):
                        def f_oc():
                            lo, hi = q * 256, q * 256 + 256
                            if act:
                                nc.scalar.copy(st["og"][:, lo:hi],
                                               ps_pv_t[:, lo:hi])
                            else:
                                nc.vector.tensor_copy(st["og"][:, lo:hi],
                                                      ps_pv_t[:, lo:hi])
                        return f_oc

                    steps = [lambda: f_ocopy(0),
                             lambda: f_rowmm_h(0), lambda: f_recip_h(0),
                             lambda: f_bc_h(0),
                             mk_ocopy_q(2, True),
                             mk_scale(0), mk_dma(0, lambda: nc.sync),
                             lambda: f_rowmm_h(1), lambda: f_recip_h(1),
                             lambda: f_bc_h(1),
                             mk_ocopy_q(3, False),
                             mk_scale_q(2), mk_dma_q(2, lambda: nc.scalar),
                             mk_scale_q(3), mk_dma_q(3, lambda: nc.sync)]
                    return steps

                fin_positions = [2, 3, 4, 5, 6, 7, 8, 9, 10]

                pT_hist = {}
                if sec == 0:
                    pT_hist[0] = pT_prev
                for mc in range(NC):
                    if not (mc == 0 and sec == 0):
                        ps_s = emit_scores(mc)
                    if mc < PVLAG and last_pv_chain[0]:
                        last_pv_chain[0][mc]()
                    if mc >= PVLAG:
                        emit_pv(mc - PVLAG, pT_hist.pop(mc - PVLAG))
                    if mc >= 1:
                        emit_denom(mc - 1, pT_hist[mc - 1])
                    if not (mc == 0 and sec == 0):
                        pT_hist[mc] = emit_exp(ps_s, mc)
                    if mc >= 1:
                        if sec == 0 and mc in setup_sched:
                            for fn in setup_sched[mc]:
                                fn()
                        if sec > 0 and fin_jobs[0]:
                            if mc in fin_positions:
                                idx = fin_positions.index(mc)
                                if idx < len(fin_jobs[0]):
                                    fin_jobs[0][idx]()
                        if sec == 1 and mc == 18:
                            emit_k_half(2, 0)
                        elif sec == 1 and mc == 22:
                            emit_k_half(2, 1)
                        elif sec == 2 and mc == 18:
                            emit_k_half(3, 0)
                        elif sec == 2 and mc == 22:
                            emit_k_half(3, 1)

                # pend the last PVLAG PVs + last denom into the next section
                def mk_last(ps=[pT_hist[NC - PVLAG + i] for i in range(PVLAG)],
                            pv=emit_pv, dn=emit_denom, sec=sec):
                    jobs = []
                    for i in range(PVLAG):
                        def run(i=i):
                            pv(NC - PVLAG + i, ps[i])
                            if i == PVLAG - 1 and sec != NSEC - 1:
                                dn(NC - 1, ps[i])
                        jobs.append(run)
                    return jobs

                last_pv_chain[0] = mk_last()
                fin_jobs[0] = make_fin(pT31=pT_hist[NC - 1])

            # drain: last section's PVs + finalize immediately
            if last_pv_chain[0] is not None:
                for fn in last_pv_chain[0]:
                    fn()
                last_pv_chain[0] = None
            for step in fin_jobs[0]:
                step()

    nc.finalize()
    return nc


def _get_nc():
    if "nc" not in _cache:
        _cache["nc"] = _build()
    return _cache["nc"]


def make_wp(Wk, Wq, Wv, bk, bq, bv):
    # layout: startup-critical wk/wq first so their DMA can land alone
    wp = np.zeros((128, 643), np.float32)
    wp[:, 0:128] = Wk.T
    wp[:, 128:256] = Wq.T
    wp[:, 256:384] = Wv.T
    wp[:, 384:512] = np.eye(128, dtype=np.float32)
    wp[:, 512] = 1.0
    wp[:, 513:641] = np.broadcast_to(bv[None, :], (128, 128))
    wp[:, 641] = bk
    wp[:, 642] = bq
    return wp


def kernel(x, Wk, bk, Wq, bq, Wv, bv, **_ignored):
    from concourse.bass_utils import run_bass_kernel_spmd

    x = np.asarray(x, dtype=np.float32)
    wp = make_wp(
        np.asarray(Wk, np.float32), np.asarray(Wq, np.float32),
        np.asarray(Wv, np.float32), np.asarray(bk, np.float32),
        np.asarray(bq, np.float32), np.asarray(bv, np.float32),
    )

    nc = _get_nc()
    import ml_dtypes
    in_maps = [
        {"xt": np.ascontiguousarray(x[b].T),
         "xb": np.ascontiguousarray(x[b].T).astype(ml_dtypes.bfloat16),
         "wp": wp} for b in range(B)
    ]
    res = run_bass_kernel_spmd(nc, in_maps, core_ids=list(range(B)))
    out = np.stack(
        [np.ascontiguousarray(res.results[b]["y"].T) for b in range(B)],
        axis=0)
    return out
